# revision 33
# baseline (speedup 1.0000x reference)
"""Trainium2 Bass kernel for nn_EnsembleSharedVQC: 12-qubit, 4-layer VQC ensemble
(4 encoders, shared theta), batch 1024, <Z_q> readout, softmax(alpha) mixture.

Sharding: pure data parallelism, 8 cores x 128 samples; theta/alpha replicated.
Everything is SBUF-resident per core (state = 128x4096 re/im f32 planes);
HBM traffic is just the tiny inputs/outputs.

Pipeline (f32 numerics throughout):
  - T-major layout S[h, b*32+u], h = (q6, q0..q5) on partitions (q6 = MSB),
    u = (q7..q11) in the free dim.
  - Per layer l, on-device-built fused unitaries (identity evolved through the
    elementwise gate machinery, CNOTs folded):
      A_l = E_hi R_hi O_hi^{l-1} (128x128 complex, hi qubits),
      B_l = E_lo R_lo O_lo^{l-1} (32x32, replicated to blockdiag4).
  - Layer 0 folded into the encoding: the state is a product state H (x) L
    until the first crossing CNOT(6,7), so evolve the factors (H1 = A0 H,
    L1 = B0 L, L1x = X7 B0 L; all tiny matmuls) and materialize
    S[h,(b,u)] = H1[h,b] * Lsel(q6)[u,b] with a q6-conditional broadcast
    combine. Skips layer-0 MMs, transposes, and evacuations entirely.
  - Layers 1..3: hi-MM and lo-MM via Gauss 3-mult complex multiply
    (M1 = Ar S_re, M2 = Ai S_im, M3 = (Ar+Ai)(S_re+S_im); re = M1-M2,
    im = M3-M1-M2), sum-plane prep per chunk on gpsimd, PSUM->SBUF combine
    as 1 ACT copy + 3 DVE subs (one PSUM input per DVE op). The lo-MM
    applies blockdiag(B) on q6=0 columns and blockdiag(X7 B) on q6=1
    columns, consuming the crossing CNOT(6,7).
  - Transposes: 4 PE 128x128 transposes share one PSUM bank -> single
    [128,512] ACT evacuation copy.
  - Last layer: |psi|^2 computed in u-major (ACT squares + DVE add), only
    the single p plane is transposed back.
  - h_angle_rx encodes to a constant state -> its whole 4-layer evolution
    runs on dedicated [128,128] tiles, emitted first so its latency hides
    under encoder-1 PE work; its measurement is deferred to the tail.
  - Measurement: only the last layer's odd CNOTs pend -> factorized hi/lo
    XOR-parities; signed marginals via sliced reduces + subtract-folds,
    per-sample outputs via one PE transpose per sign pattern.
  - softmax(alpha) mixing on device; out = [128, 12] f32 per core.
"""

import numpy as np

import concourse.bass as bass
import concourse.bacc as bacc
import concourse.mybir as mybir
import concourse.tile as tile
from concourse.bass_utils import run_bass_kernel_spmd

AF = mybir.ActivationFunctionType
OP = mybir.AluOpType
F32 = mybir.dt.float32

N = 12
DIM = 1 << N
LAYERS = 4
B_CORE = 128
N_CORES = 8
ENCODERS = ["angle_rx", "angle_ry", "h_angle_rx", "h_angle_ry"]
INV_SQRT2 = float(1.0 / np.sqrt(2.0))
HALF_PI = float(np.pi / 2.0)

HI_ORDER = [6, 0, 1, 2, 3, 4, 5]
LO_ORDER = [7, 8, 9, 10, 11]
E_HI = [(0, 1), (2, 3), (4, 5)]
O_HI = [(1, 2), (3, 4), (5, 6)]
E_LO = [(8, 9), (10, 11)]
O_LO = [(7, 8), (9, 10)]


def _bitview(ap, nbits, fixed):
    """View a [P, 2**nbits] plane with some bit positions fixed."""
    names = [f"b{i}" for i in range(nbits)]
    pat = "p ({}) -> p {}".format(" ".join(names), " ".join(names))
    v = ap.rearrange(pat, **{n: 2 for n in names[:-1]})
    idx = [slice(None)] * (nbits + 1)
    for pos, val in fixed.items():
        idx[1 + pos] = slice(val, val + 1)
    return v[tuple(idx)]


def _small_gate(nc, cur, nxt, nbits, pos, rev, cw, npart):
    """Fused SU(2) gate on wire `pos` of an npart x 2**nbits state."""
    pr, pi = cur
    qr, qi = nxt
    cw = {k: v[:npart] for k, v in cw.items()}
    combos = [()]
    for _ in rev:
        combos = [c + (v,) for c in combos for v in (0, 1)]
    pr0 = _bitview(pr, nbits, {pos: 0})[:npart]
    pi0 = _bitview(pi, nbits, {pos: 0})[:npart]
    stt = nc.vector.scalar_tensor_tensor
    for out_pl, a, c1, c2, c3, c4 in (
        (qr, 0, cw['w'], cw['nx'], cw['y'], cw['nz']),
        (qi, 0, cw['x'], cw['w'], cw['z'], cw['y']),
        (qr, 1, cw['ny'], cw['nz'], cw['w'], cw['x']),
        (qi, 1, cw['z'], cw['ny'], cw['nx'], cw['w']),
    ):
        o_full = _bitview(out_pl, nbits, {pos: a})[:npart]
        nc.scalar.mul(o_full, pr0, c1)
        stt(o_full, pi0, c2, o_full, op0=OP.mult, op1=OP.add)
        for combo in combos:
            ofix = {pos: a}
            ifix = {pos: 1}
            for rp, v in zip(rev, combo):
                ofix[rp] = v
                ifix[rp] = 1 - v
            o_p = _bitview(out_pl, nbits, ofix)[:npart]
            pr1 = _bitview(pr, nbits, ifix)[:npart]
            pi1 = _bitview(pi, nbits, ifix)[:npart]
            stt(o_p, pr1, c3, o_p, op0=OP.mult, op1=OP.add)
            stt(o_p, pi1, c4, o_p, op0=OP.mult, op1=OP.add)


def _small_swap(nc, planes, nbits, c_pos, t_pos, tmp, npart):
    """Physical CNOT(c_pos -> t_pos) swap on an npart x 2**nbits state."""
    qdim = 1 << (nbits - 2)
    for pl in planes:
        v0 = _bitview(pl, nbits, {c_pos: 1, t_pos: 0})[:npart]
        v1 = _bitview(pl, nbits, {c_pos: 1, t_pos: 1})[:npart]
        t = tmp[:npart, 0:qdim]
        nc.vector.tensor_copy(t, v0)
        nc.scalar.copy(v0, v1)
        nc.vector.tensor_copy(v1, t)


def build_nc_stage3():
    nc = bacc.Bacc(None, target_bir_lowering=False, debug=False)

    features = nc.dram_tensor("features", [B_CORE, N], F32, kind="ExternalInput").ap()
    theta = nc.dram_tensor("theta", [LAYERS, N, 3], F32, kind="ExternalInput").ap()
    alpha = nc.dram_tensor("alpha", [4], F32, kind="ExternalInput").ap()
    out = nc.dram_tensor("out", [B_CORE, N], F32, kind="ExternalOutput").ap()

    P = B_CORE
    AX = mybir.AxisListType

    from concourse.masks import make_identity

    with tile.TileContext(nc) as tc:
        with (
            tc.tile_pool(name="state", bufs=1) as sp,
            tc.tile_pool(name="small", bufs=1) as small,
            tc.tile_pool(name="scratch", bufs=4) as scratch,
            tc.tile_pool(name="mm", bufs=6, space="PSUM") as mmpool,
            tc.tile_pool(name="tp", bufs=2, space="PSUM") as tppool,
            tc.tile_pool(name="dram", bufs=8, space="DRAM") as dpool,
        ):
            # ---------- input DMA ----------
            feat = small.tile([P, N], F32, tag="feat", name="feat")
            nc.sync.dma_start(out=feat, in_=features)
            th = small.tile([P, LAYERS, N, 3], F32, tag="th", name="th")
            th_b = bass.AP(tensor=theta.tensor, offset=0,
                           ap=[[0, P], [N * 3, LAYERS], [3, N], [1, 3]])
            nc.sync.dma_start(out=th, in_=th_b)
            alp = small.tile([P, 4], F32, tag="alp", name="alp")
            alp_b = bass.AP(tensor=alpha.tensor, offset=0, ap=[[0, P], [1, 4]])
            nc.sync.dma_start(out=alp, in_=alp_b)

            # ---------- trig ----------
            hpi = small.tile([P, 1], F32, tag="hpi", name="hpi")
            nc.vector.memset(hpi, HALF_PI)
            fh = small.tile([P, N], F32, tag="fh", name="fh")
            nc.scalar.activation(fh, feat, AF.Copy, scale=0.5)
            cf = small.tile([P, N], F32, tag="cf", name="cf")
            nc.scalar.activation(cf, fh, AF.Sin, bias=hpi)
            sf = small.tile([P, N], F32, tag="sf", name="sf")
            nc.scalar.activation(sf, fh, AF.Sin)
            nsf = small.tile([P, N], F32, tag="nsf", name="nsf")
            nc.vector.tensor_scalar_mul(nsf, sf, -1.0)
            hc = small.tile([P, N], F32, tag="hc", name="hc")
            nc.vector.tensor_sub(hc, cf, sf)
            nc.vector.tensor_scalar_mul(hc, hc, INV_SQRT2)
            hs = small.tile([P, N], F32, tag="hs", name="hs")
            nc.vector.tensor_add(hs, cf, sf)
            nc.vector.tensor_scalar_mul(hs, hs, INV_SQRT2)

            def flat(ap):
                return ap.rearrange("p a b c -> p (a b c)")

            thh = small.tile([P, LAYERS, N, 3], F32, tag="thh", name="thh")
            nc.scalar.activation(flat(thh), flat(th), AF.Copy, scale=0.5)
            ct = small.tile([P, LAYERS, N, 3], F32, tag="ct", name="ct")
            nc.scalar.activation(flat(ct), flat(thh), AF.Sin, bias=hpi)
            st = small.tile([P, LAYERS, N, 3], F32, tag="st", name="st")
            nc.scalar.activation(flat(st), flat(thh), AF.Sin)

            ca, cb, cg = (ct[:, :, :, i:i + 1] for i in range(3))
            sa, sb, sg = (st[:, :, :, i:i + 1] for i in range(3))

            def lq_tile(tag):
                return small.tile([P, LAYERS, N, 1], F32, tag=tag, name=tag)

            t1, t2, t3, t4 = (lq_tile(f"t{i}") for i in range(4))
            nc.vector.tensor_mul(t1, cg, cb)
            nc.vector.tensor_mul(t2, sg, sb)
            nc.vector.tensor_mul(t3, cg, sb)
            nc.vector.tensor_mul(t4, sg, cb)
            u1, u2 = lq_tile("u1"), lq_tile("u2")
            w_c, x_c, y_c, z_c = (lq_tile(t) for t in ("w", "x", "y", "z"))
            nx_c, ny_c, nz_c = (lq_tile(t) for t in ("nx", "ny", "nz"))
            nc.vector.tensor_mul(u1, t1, ca)
            nc.vector.tensor_mul(u2, t2, sa)
            nc.vector.tensor_add(w_c, u1, u2)
            nc.vector.tensor_mul(u1, t3, sa)
            nc.vector.tensor_mul(u2, t4, ca)
            nc.vector.tensor_sub(x_c, u1, u2)
            nc.vector.tensor_scalar_mul(nx_c, x_c, -1.0)
            nc.vector.tensor_mul(u1, t3, ca)
            nc.vector.tensor_mul(u2, t4, sa)
            nc.vector.tensor_add(ny_c, u1, u2)
            nc.vector.tensor_scalar_mul(y_c, ny_c, -1.0)
            nc.vector.tensor_mul(u1, t2, ca)
            nc.vector.tensor_mul(u2, t1, sa)
            nc.vector.tensor_sub(z_c, u1, u2)
            nc.vector.tensor_scalar_mul(nz_c, z_c, -1.0)

            def coefs(l, q):
                return {k: c[:, l:l + 1, q:q + 1, :] for k, c in
                        (('w', w_c), ('x', x_c), ('y', y_c), ('z', z_c),
                         ('nx', nx_c), ('ny', ny_c), ('nz', nz_c))}

            # ---------- softmax(alpha) ----------
            amax = small.tile([P, 1], F32, tag="amax", name="amax")
            nc.vector.reduce_max(amax, alp, axis=AX.X)
            esh = small.tile([P, 4], F32, tag="esh", name="esh")
            nc.vector.tensor_scalar(esh, alp, amax, None, op0=OP.subtract)
            nc.scalar.activation(esh, esh, AF.Exp)
            ssum = small.tile([P, 1], F32, tag="ssum", name="ssum")
            nc.vector.reduce_sum(ssum, esh, axis=AX.X)
            rsum = small.tile([P, 1], F32, tag="rsum", name="rsum")
            nc.vector.reciprocal(rsum, ssum)
            wts = small.tile([P, 4], F32, tag="wts", name="wts")
            nc.vector.tensor_scalar(wts, esh, rsum, None, op0=OP.mult)

            # ---------- identity for PE transposes ----------
            ident = small.tile([P, P], F32, tag="ident", name="ident")
            make_identity(nc, ident)

            # ---------- build A_l (hi) and B_l (lo) stationaries ----------
            bre_a = small.tile([P, P], F32, tag="bre_a", name="bre_a")
            bim_a = small.tile([P, P], F32, tag="bim_a", name="bim_a")
            bre_b = small.tile([P, P], F32, tag="bre_b", name="bre_b")
            bim_b = small.tile([P, P], F32, tag="bim_b", name="bim_b")
            btmp = small.tile([P, 64], F32, tag="btmp", name="btmp")

            A_t = []   # per layer: (Ar, Ai, Aq=Ar+Ai) [128,128] lhsT (= A^T)
            B_t = []   # per layer: (Br, Bi, Bq, BrX, BiX, BqX) blockdiag lhsT
            sB0 = {}   # layer-0 32x32 lo lhsT for the factorized layer 0

            def build_small(order, qubits, flips_pre, flips_post, l, npart):
                nbits = {7: 7, 5: 5}[len(order)]
                dim = 1 << nbits
                make_identity(nc, bre_a[:dim, :dim])
                nc.gpsimd.memset(bim_a[:, 0:dim], 0.0)
                cur = (bre_a[:, 0:dim], bim_a[:, 0:dim])
                nxt = (bre_b[:, 0:dim], bim_b[:, 0:dim])
                pend = [(order.index(c), order.index(t)) for c, t in flips_pre]
                for q in qubits:
                    pos = order.index(q)
                    rev = [t for c_, t in pend if c_ == pos]
                    pend = [(c_, t) for c_, t in pend if c_ != pos]
                    _small_gate(nc, cur, nxt, nbits, pos, rev, coefs(l, q), npart)
                    cur, nxt = nxt, cur
                assert not pend
                for c_, t in flips_post:
                    _small_swap(nc, cur, nbits, order.index(c_),
                                order.index(t), btmp, npart)
                return cur

            for l in range(LAYERS):
                ohi = O_HI if l > 0 else []
                olo = O_LO if l > 0 else []
                curA = build_small(HI_ORDER, [0, 1, 2, 3, 4, 5, 6], ohi, E_HI, l, P)
                Ar = small.tile([P, P], F32, tag=f"Ar{l}", name=f"Ar{l}")
                Ai = small.tile([P, P], F32, tag=f"Ai{l}", name=f"Ai{l}")
                Aq = small.tile([P, P], F32, tag=f"Aq{l}", name=f"Aq{l}")
                nc.vector.tensor_copy(Ar, curA[0])
                nc.vector.tensor_copy(Ai, curA[1])
                nc.vector.tensor_add(Aq, curA[0], curA[1])
                A_t.append((Ar, Ai, Aq))

                curB = build_small(LO_ORDER, [7, 8, 9, 10, 11], olo, E_LO, l, 32)
                sBr = small.tile([32, 32], F32, tag=f"sBr{l}", name=f"sBr{l}")
                sBi = small.tile([32, 32], F32, tag=f"sBi{l}", name=f"sBi{l}")
                sBq = small.tile([32, 32], F32, tag=f"sBq{l}", name=f"sBq{l}")
                nc.vector.tensor_copy(sBr, curB[0][:32, :32])
                nc.vector.tensor_copy(sBi, curB[1][:32, :32])
                nc.vector.tensor_add(sBq, curB[0][:32, :32], curB[1][:32, :32])
                if l == 0:
                    sBxr = small.tile([32, 32], F32, tag="sBxr", name="sBxr")
                    sBxi = small.tile([32, 32], F32, tag="sBxi", name="sBxi")
                    for dst, src in ((sBxr, sBr), (sBxi, sBi)):
                        nc.vector.tensor_copy(
                            dst.rearrange("p (t r) -> p t r", t=2),
                            src.rearrange("p (t r) -> p t r", t=2)[:, ::-1, :])
                    sB0 = {"r": sBr, "i": sBi, "xr": sBxr, "xi": sBxi}
                names = (f"Br{l}", f"Bi{l}", f"Bq{l}",
                         f"BrX{l}", f"BiX{l}", f"BqX{l}")
                tiles = []
                for nm in names:
                    tt = small.tile([P, P], F32, tag=nm, name=nm)
                    nc.gpsimd.memset(tt, 0.0)
                    tiles.append(tt)
                Br, Bi, Bq, BrX, BiX, BqX = tiles
                for i in range(4):
                    sl = slice(32 * i, 32 * i + 32)
                    for dst, src in ((Br, sBr), (Bi, sBi), (Bq, sBq)):
                        nc.sync.dma_start(out=dst[sl, sl], in_=src)
                    for dst, src in ((BrX, sBr), (BiX, sBi), (BqX, sBq)):
                        swp = dst[sl, sl].rearrange("p (t r) -> p t r", t=2)
                        nc.sync.dma_start(
                            out=swp, in_=src.rearrange(
                                "p (t r) -> p t r", t=2)[:, ::-1, :])
                B_t.append((Br, Bi, Bq, BrX, BiX, BqX))

            # ---------- state planes ----------
            S_re = sp.tile([P, DIM], F32, tag="S_re", name="S_re")
            S_im = sp.tile([P, DIM], F32, tag="S_im", name="S_im")
            T_re = sp.tile([P, DIM], F32, tag="T_re", name="T_re")
            T_im = sp.tile([P, DIM], F32, tag="T_im", name="T_im")
            U_re = sp.tile([P, DIM], F32, tag="U_re", name="U_re")
            U_im = sp.tile([P, DIM], F32, tag="U_im", name="U_im")
            V_re = sp.tile([P, DIM], F32, tag="V_re", name="V_re")
            V_im = sp.tile([P, DIM], F32, tag="V_im", name="V_im")
            Ssum = sp.tile([P, DIM], F32, tag="Ssum", name="Ssum")

            # encoding scratch
            Hb_re = small.tile([P, P], F32, tag="Hb_re", name="Hb_re")
            Hb_im = small.tile([P, P], F32, tag="Hb_im", name="Hb_im")
            H_re = small.tile([P, P], F32, tag="H_re", name="H_re")
            H_im = small.tile([P, P], F32, tag="H_im", name="H_im")
            H1_re = small.tile([P, P], F32, tag="H1_re", name="H1_re")
            H1_im = small.tile([P, P], F32, tag="H1_im", name="H1_im")
            Lb_re = small.tile([P, 32], F32, tag="Lb_re", name="Lb_re")
            Lb_im = small.tile([P, 32], F32, tag="Lb_im", name="Lb_im")
            Lu_re = small.tile([32, P], F32, tag="Lu_re", name="Lu_re")
            Lu_im = small.tile([32, P], F32, tag="Lu_im", name="Lu_im")
            L1 = {}
            for nm in ("re", "im", "xre", "xim"):
                L1[nm] = small.tile([P, 32], F32, tag=f"L1{nm}", name=f"L1{nm}")

            zacc = small.tile([P, N], F32, tag="zacc", name="zacc")
            nc.vector.memset(zacc, 0.0)

            def doubling(re_t, im_t, order, enc, cplx):
                nc.vector.memset(re_t[:, 0:1], 1.0)
                if cplx:
                    nc.gpsimd.memset(im_t, 0.0)
                size = 1
                for q in reversed(order):
                    lo = re_t[:, 0:size]
                    hi = re_t[:, size:2 * size]
                    if enc == "angle_rx":
                        loi = im_t[:, 0:size]
                        hii = im_t[:, size:2 * size]
                        v0 = cf[:, q:q + 1]
                        nc.scalar.mul(hi, loi, sf[:, q:q + 1])
                        nc.scalar.mul(hii, lo, nsf[:, q:q + 1])
                        nc.scalar.mul(loi, loi, v0)
                        nc.scalar.mul(lo, lo, v0)
                    else:
                        if enc == "angle_ry":
                            a_ap, b_ap = cf[:, q:q + 1], sf[:, q:q + 1]
                        else:
                            a_ap, b_ap = hc[:, q:q + 1], hs[:, q:q + 1]
                        nc.scalar.mul(hi, lo, b_ap)
                        nc.scalar.mul(lo, lo, a_ap)
                    size *= 2

            def u_major_view(plane, g0, ng):
                """[p, 2(q6), ng, 64] view of U-major cols, groups g0..g0+ng."""
                v = plane.rearrange("p (s g2 h) -> p s g2 h", s=2, g2=32)
                return v[:, :, g0:g0 + ng, :]

            def gauss_mm(dst_re, dst_im, Gr, Gi, Gq, src_re, src_im, src_q,
                         width, mview=None):
                """dst = G @ src (complex) via Gauss; dst views [128,width].
                mview reshapes the PSUM banks to match scattered dst views."""
                m1 = mmpool.tile([P, width], F32, tag="mm", name="m1")
                m2 = mmpool.tile([P, width], F32, tag="mm", name="m2")
                m3 = mmpool.tile([P, width], F32, tag="mm", name="m3")
                nc.tensor.matmul(m1, Gr, src_re, start=True, stop=True)
                nc.tensor.matmul(m2, Gi, src_im, start=True, stop=True)
                nc.tensor.matmul(m3, Gq, src_q, start=True, stop=True)
                v1, v2, v3 = ((mview(m1), mview(m2), mview(m3)) if mview
                              else (m1, m2, m3))
                # only one PSUM input per DVE op: stage M1 into dst_re first
                nc.scalar.copy(dst_re, v1)
                nc.vector.tensor_sub(dst_im, v3, dst_re)
                nc.vector.tensor_sub(dst_im, dst_im, v2)
                nc.vector.tensor_sub(dst_re, dst_re, v2)

            def measure(enc_i, p_t):
                """<Z_q> of the pending-flip-factorized p plane -> zacc."""
                LO_PATS = {0: (), 1: (0,), 2: (0, 1), 3: (2,), 4: (2, 3),
                           5: (4,)}
                r_pats = {}
                for pid, bits in LO_PATS.items():
                    eng = nc.vector
                    if not bits:
                        r = small.tile([P, P], F32, tag=f"rpat{pid}",
                                       name=f"rp{pid}")
                        eng.reduce_sum(
                            r, p_t.rearrange("p (b u) -> p b u", u=32),
                            axis=AX.X)
                        r_pats[pid] = r
                        continue
                    a0, b0 = bits[0], bits[-1]
                    runw = 1 << (b0 - a0 + 1)
                    o_sz = 1 << a0
                    i_sz = 32 // (o_sz * runw)
                    w4 = scratch.tile([P, P * runw], F32, tag="w4",
                                      name=f"w4_{pid}")
                    if o_sz == 1 and i_sz > 1:
                        vv = p_t.rearrange("p (b t i) -> p b t i", b=P, t=runw)
                        eng.reduce_sum(w4, vv, axis=AX.X)
                    elif i_sz == 1 and o_sz > 1:
                        vv = p_t.rearrange("p (b o t) -> p b t o", b=P, t=runw)
                        eng.reduce_sum(w4, vv, axis=AX.X)
                    else:
                        vv = p_t.rearrange("p (b o t i) -> p b t o i",
                                           b=P, o=o_sz, t=runw)
                        eng.reduce_sum(w4, vv, axis=AX.XY)
                    src, width = w4, runw
                    while width > 1:
                        width //= 2
                        dst = (scratch.tile([P, P * width], F32, tag="fold2",
                                            name="fold2")
                               if width > 1 else
                               small.tile([P, P], F32, tag=f"rpat{pid}",
                                          name=f"rpf{pid}"))
                        s2 = src.rearrange("p (b t) -> p b t", t=2 * width)
                        eng.tensor_sub(
                            dst.rearrange("p (b t) -> p b t", t=width),
                            s2[:, :, 0:width], s2[:, :, width:2 * width])
                        src = dst
                    r_pats[pid] = src

                rT = {}
                for pid, r in r_pats.items():
                    ptr = tppool.tile([P, 512], F32, tag="tp", name="ptr")
                    nc.tensor.transpose(ptr[:, 0:P], r, ident)
                    rt = small.tile([P, P], F32, tag=f"rT{pid}",
                                    name=f"rT{pid}")
                    nc.scalar.copy(rt, ptr[:, 0:P])
                    rT[pid] = rt

                z_e = small.tile([P, N], F32, tag=f"z_e{enc_i}",
                                 name=f"z_e{enc_i}")
                T_HI = {0: [0], 1: [1], 2: [1, 2], 3: [3], 4: [3, 4], 5: [5],
                        6: [5, 6]}
                for q in range(N):
                    if q <= 6:
                        src_m = rT[0]
                        bits = sorted(HI_ORDER.index(b2) for b2 in T_HI[q])
                        nb = 7
                    else:
                        pid = {7: 1, 8: 2, 9: 3, 10: 4, 11: 5}[q]
                        src_m = rT[pid]
                        bits = []
                        nb = 7
                    enq = nc.vector
                    if not bits:
                        enq.reduce_sum(z_e[:, q:q + 1], src_m, axis=AX.X)
                        continue
                    rest = [i for i in range(nb) if i not in bits]
                    runs = []
                    for i in rest:
                        if runs and runs[-1][-1] == i - 1:
                            runs[-1].append(i)
                        else:
                            runs.append([i])
                    assert len(runs) <= 2, (bits, runs)
                    names = [f"h{i}" for i in range(nb)]
                    pat = "p ({}) -> p {} {}".format(
                        " ".join(names),
                        " ".join(names[i] for i in bits),
                        " ".join("(" + " ".join(names[j] for j in run) + ")"
                                 for run in runs))
                    vv = src_m.rearrange(pat, **{n: 2 for n in names[:-1]})
                    kw = 1 << len(bits)
                    wq = scratch.tile([P, kw], F32, tag="wq", name="wq")
                    enq.reduce_sum(
                        wq, vv, axis=AX.X if len(runs) == 1 else AX.XY)
                    srcf, width = wq, kw
                    while width > 1:
                        width //= 2
                        dstf = (z_e[:, q:q + 1] if width == 1 else
                                scratch.tile([P, width], F32, tag="foldq",
                                             name="foldq"))
                        enq.tensor_sub(dstf, srcf[:, 0:width],
                                       srcf[:, width:2 * width])
                        srcf = dstf
                nc.vector.scalar_tensor_tensor(
                    zacc, z_e, wts[:, enc_i:enc_i + 1], zacc,
                    op0=OP.mult, op1=OP.add)

            # ---------- h_angle_rx branch on dedicated small tiles ----------
            # Sample-independent constant state: evolve one 128-col block.
            # Emitted first so its latency hides under encoder-1 PE work;
            # measurement is deferred to the tail.
            f3 = {nm: small.tile([P, 128], F32, tag=f"f3{nm}", name=f"f3{nm}")
                  for nm in ("Sre", "Sim", "Tre", "Tim", "Ure", "Uim",
                             "Vre", "Vim", "sum")}
            nc.vector.memset(f3["Sre"], float(2.0 ** -6))
            nc.gpsimd.memset(f3["Sim"], 0.0)
            for l in range(LAYERS):
                Ar, Ai, Aq = A_t[l]
                Br, Bi, Bq, BrX, BiX, BqX = B_t[l]
                nc.gpsimd.tensor_add(f3["sum"], f3["Sre"], f3["Sim"])
                gauss_mm(f3["Tre"], f3["Tim"], Ar, Ai, Aq,
                         f3["Sre"], f3["Sim"], f3["sum"], 128)
                for nin, nout in (("Tre", "Ure"), ("Tim", "Uim")):
                    ptf = tppool.tile([P, 512], F32, tag="tp", name="ptf3")
                    nc.tensor.transpose(ptf[:, 0:P], f3[nin], ident)
                    nc.scalar.copy(f3[nout], ptf[:, 0:P])
                nc.gpsimd.tensor_add(f3["sum"][:, 0:64], f3["Ure"][:, 0:64],
                                     f3["Uim"][:, 0:64])
                nc.gpsimd.tensor_add(f3["sum"][:, 64:128],
                                     f3["Ure"][:, 64:128],
                                     f3["Uim"][:, 64:128])
                for (ucs, br, bi, bq) in (
                    (slice(0, 64), Br, Bi, Bq),
                    (slice(64, 128), BrX, BiX, BqX),
                ):
                    gauss_mm(f3["Vre"][:, ucs], f3["Vim"][:, ucs], br, bi, bq,
                             f3["Ure"][:, ucs], f3["Uim"][:, ucs],
                             f3["sum"][:, ucs], 64)
                for nin, nout in (("Vre", "Sre"), ("Vim", "Sim")):
                    ptb = tppool.tile([P, 512], F32, tag="tp", name="ptb3")
                    nc.tensor.transpose(ptb[:, 0:P], f3[nin], ident)
                    nc.scalar.copy(f3[nout], ptb[:, 0:P])

            # ---------- the three data-dependent encoders ----------
            for enc, enc_i in (("angle_rx", 0), ("angle_ry", 1),
                               ("h_angle_ry", 3)):
                # ---------- factors + layer 0 ----------
                cplx = enc == "angle_rx"
                doubling(Hb_re, Hb_im, HI_ORDER, enc, cplx)
                doubling(Lb_re, Lb_im, LO_ORDER, enc, cplx)
                # Hb -> H (h-major)
                pt = tppool.tile([P, 512], F32, tag="tp", name="ptH")
                nc.tensor.transpose(pt[:, 0:P], Hb_re, ident)
                nc.scalar.copy(H_re, pt[:, 0:P])
                if cplx:
                    pt2 = tppool.tile([P, 512], F32, tag="tp", name="ptH2")
                    nc.tensor.transpose(pt2[:, 0:P], Hb_im, ident)
                    nc.scalar.copy(H_im, pt2[:, 0:P])
                # H1 = A0 @ H
                Ar0, Ai0, _Aq0 = A_t[0]
                mre = mmpool.tile([P, 512], F32, tag="mm", name="mre")
                mim = mmpool.tile([P, 512], F32, tag="mm", name="mim")
                if cplx:
                    m2b = mmpool.tile([P, 512], F32, tag="mm", name="m2b")
                    nc.tensor.matmul(mre[:, 0:P], Ar0, H_re,
                                     start=True, stop=True)
                    nc.tensor.matmul(m2b[:, 0:P], Ai0, H_im,
                                     start=True, stop=True)
                    nc.scalar.copy(H1_re, mre[:, 0:P])
                    nc.vector.tensor_sub(H1_re, H1_re, m2b[:, 0:P])
                    nc.tensor.matmul(mim[:, 0:P], Ar0, H_im,
                                     start=True, stop=False)
                    nc.tensor.matmul(mim[:, 0:P], Ai0, H_re,
                                     start=False, stop=True)
                    nc.vector.tensor_copy(H1_im, mim[:, 0:P])
                else:
                    nc.tensor.matmul(mre[:, 0:P], Ar0, H_re,
                                     start=True, stop=True)
                    nc.tensor.matmul(mim[:, 0:P], Ai0, H_re,
                                     start=True, stop=True)
                    nc.scalar.copy(H1_re, mre[:, 0:P])
                    nc.vector.tensor_copy(H1_im, mim[:, 0:P])
                # Lb -> Lu (u-major)
                ptl = tppool.tile([P, 512], F32, tag="tp", name="ptL")
                nc.tensor.transpose(ptl[:32, 0:P], Lb_re, ident)
                nc.scalar.copy(Lu_re, ptl[:32, 0:P])
                if cplx:
                    ptl2 = tppool.tile([P, 512], F32, tag="tp", name="ptL2")
                    nc.tensor.transpose(ptl2[:32, 0:P], Lb_im, ident)
                    nc.scalar.copy(Lu_im, ptl2[:32, 0:P])
                # L1b = (B0 L)^T, L1xb = (X7 B0 L)^T, directly b-major:
                # matmul(lhsT=Lu, rhs=sB0) = Lu.T @ B0^T = Lb @ B0^T
                for pre, keyr, keyi in (("", "r", "i"), ("x", "xr", "xi")):
                    br, bi = sB0[keyr], sB0[keyi]
                    ma = mmpool.tile([P, 512], F32, tag="mm", name="ma")
                    mb = mmpool.tile([P, 512], F32, tag="mm", name="mb")
                    if cplx:
                        mc = mmpool.tile([P, 512], F32, tag="mm", name="mc")
                        nc.tensor.matmul(ma[:, 0:32], Lu_re, br,
                                         start=True, stop=True)
                        nc.tensor.matmul(mc[:, 0:32], Lu_im, bi,
                                         start=True, stop=True)
                        nc.scalar.copy(L1[pre + "re"], ma[:, 0:32])
                        nc.vector.tensor_sub(L1[pre + "re"],
                                             L1[pre + "re"], mc[:, 0:32])
                        nc.tensor.matmul(mb[:, 0:32], Lu_im, br,
                                         start=True, stop=False)
                        nc.tensor.matmul(mb[:, 0:32], Lu_re, bi,
                                         start=False, stop=True)
                        nc.vector.tensor_copy(L1[pre + "im"], mb[:, 0:32])
                    else:
                        nc.tensor.matmul(ma[:, 0:32], Lu_re, br,
                                         start=True, stop=True)
                        nc.tensor.matmul(mb[:, 0:32], Lu_re, bi,
                                         start=True, stop=True)
                        nc.scalar.copy(L1[pre + "re"], ma[:, 0:32])
                        nc.vector.tensor_copy(L1[pre + "im"], mb[:, 0:32])
                # broadcast Lsel over partitions via DRAM roundtrip:
                # staging planes Ssum (re) / V_im (im) are free here;
                # dest[p=(q6,h6), (b,u)] = (L1 if q6=0 else L1x)[u, b]
                for nm, plane, half in (("re", Ssum, 0), ("im", V_im, 0),
                                        ("xre", Ssum, 1), ("xim", V_im, 1)):
                    dl = dpool.tile([P, 32], F32, tag=f"dl{nm}{half}",
                                    name=f"dl{nm}{half}")
                    nc.sync.dma_start(out=dl, in_=L1[nm])
                    rd = bass.AP(tensor=dl.tensor, offset=dl.offset,
                                 ap=[[0, 64], [1, DIM]])
                    nc.sync.dma_start(
                        out=plane[64 * half:64 * half + 64], in_=rd)

                # combine: S = hview(H1) * Lbc (complex); V_re as temp
                def hview(hp):
                    return hp.unsqueeze(2).broadcast_to((P, P, 32))
                sv_re = S_re.rearrange("p (b u) -> p b u", u=32)
                sv_im = S_im.rearrange("p (b u) -> p b u", u=32)
                lv_re = Ssum.rearrange("p (b u) -> p b u", u=32)
                lv_im = V_im.rearrange("p (b u) -> p b u", u=32)
                tv_re = V_re.rearrange("p (b u) -> p b u", u=32)
                nc.vector.tensor_tensor(sv_re, hview(H1_re), lv_re,
                                        op=OP.mult)
                nc.vector.tensor_tensor(tv_re, hview(H1_im), lv_im,
                                        op=OP.mult)
                nc.vector.tensor_sub(sv_re, sv_re, tv_re)
                nc.vector.tensor_tensor(sv_im, hview(H1_re), lv_im,
                                        op=OP.mult)
                nc.vector.tensor_tensor(tv_re, hview(H1_im), lv_re,
                                        op=OP.mult)
                nc.vector.tensor_add(sv_im, sv_im, tv_re)

                # ---------- layers 1..3 ----------
                for l in range(1, LAYERS):
                    lastp = l == LAYERS - 1
                    Ar, Ai, Aq = A_t[l]
                    Br, Bi, Bq, BrX, BiX, BqX = B_t[l]
                    # hi-MM: T = A @ S (Gauss); per-chunk sum prep
                    for c in range(8):
                        cs = slice(512 * c, 512 * c + 512)
                        nc.gpsimd.tensor_add(Ssum[:, cs], S_re[:, cs],
                                             S_im[:, cs])
                        gauss_mm(T_re[:, cs], T_im[:, cs], Ar, Ai, Aq,
                                 S_re[:, cs], S_im[:, cs], Ssum[:, cs], 512)
                    # transpose T -> U (batches of 4 into one PSUM bank)
                    for g0 in range(0, 32, 4):
                        for pl_in, pl_out in ((T_re, U_re), (T_im, U_im)):
                            pt4 = tppool.tile([P, 512], F32, tag="tp",
                                              name="pt4")
                            for j in range(4):
                                gs = slice(128 * (g0 + j), 128 * (g0 + j) + 128)
                                nc.tensor.transpose(
                                    pt4[:, 128 * j:128 * j + 128],
                                    pl_in[:, gs], ident)
                            nc.scalar.copy(
                                u_major_view(pl_out, g0, 4),
                                pt4.rearrange("p (j s h) -> p s j h",
                                              j=4, s=2))
                    # lo-MM: V = blockdiag(B or X7 B) @ U (Gauss)
                    for c in range(8):
                        cs = slice(512 * c, 512 * c + 512)
                        nc.gpsimd.tensor_add(Ssum[:, cs], U_re[:, cs],
                                             U_im[:, cs])
                        br, bi, bq = (Br, Bi, Bq) if c < 4 else (BrX, BiX, BqX)
                        q6c = c // 4
                        gb = 8 * (c % 4)

                        def v_scat(plane):
                            v = plane.rearrange("p (g s h) -> p g s h",
                                                g=32, s=2)
                            return v[:, gb:gb + 8, q6c:q6c + 1, :]

                        def mvw(m):
                            return m.rearrange("p (g s h) -> p g s h",
                                               g=8, s=1)
                        gauss_mm(v_scat(V_re), v_scat(V_im), br, bi, bq,
                                 U_re[:, cs], U_im[:, cs], Ssum[:, cs], 512,
                                 mview=mvw)
                    if lastp:
                        # squares in u-major; transpose only the p plane
                        nc.scalar.activation(U_re, V_re, AF.Square)
                        nc.scalar.activation(U_im, V_im, AF.Square)
                        nc.vector.tensor_add(U_re, U_re, U_im)
                        for g0 in range(0, 32, 4):
                            pt4 = tppool.tile([P, 512], F32, tag="tp",
                                              name="pt4p")
                            for j in range(4):
                                gs = slice(128 * (g0 + j),
                                           128 * (g0 + j) + 128)
                                nc.tensor.transpose(
                                    pt4[:, 128 * j:128 * j + 128],
                                    U_re[:, gs], ident)
                            nc.scalar.copy(T_re[:, 512 * (g0 // 4):
                                                512 * (g0 // 4) + 512], pt4)
                        continue
                    # transpose back V -> S (batches of 4)
                    for g0 in range(0, 32, 4):
                        for pl_in, pl_out in ((V_re, S_re), (V_im, S_im)):
                            pt4 = tppool.tile([P, 512], F32, tag="tp",
                                              name="pt4b")
                            for j in range(4):
                                gs = slice(128 * (g0 + j), 128 * (g0 + j) + 128)
                                nc.tensor.transpose(
                                    pt4[:, 128 * j:128 * j + 128],
                                    pl_in[:, gs], ident)
                            nc.scalar.copy(
                                pl_out[:, 512 * (g0 // 4):
                                       512 * (g0 // 4) + 512], pt4)

                # ---------- measurement ----------
                measure(enc_i, T_re)  # p plane written by the lastp branch

            # ---------- deferred h_angle_rx measurement ----------
            f3p = small.tile([P, 32], F32, tag="f3p", name="f3p")
            f3q = small.tile([P, 32], F32, tag="f3q", name="f3q")
            nc.scalar.activation(f3p, f3["Sre"][:, 0:32], AF.Square)
            nc.scalar.activation(f3q, f3["Sim"][:, 0:32], AF.Square)
            nc.vector.tensor_add(f3p, f3p, f3q)
            nc.vector.tensor_copy(
                T_re.rearrange("p (b u) -> p b u", u=32),
                f3p.unsqueeze(1).broadcast_to((P, 128, 32)))
            measure(2, T_re)

            nc.sync.dma_start(out=out, in_=zacc)

    nc.finalize()
    return nc


_NC_CACHE = None
LAST_RESULTS = None  # BassKernelResults of the most recent run (for profiling)


def kernel(features: np.ndarray, theta: np.ndarray, alpha: np.ndarray) -> np.ndarray:
    global _NC_CACHE, LAST_RESULTS
    if _NC_CACHE is None:
        _NC_CACHE = build_nc_stage3()
    nc = _NC_CACHE

    features = np.ascontiguousarray(features, dtype=np.float32)
    theta = np.ascontiguousarray(theta, dtype=np.float32)
    alpha = np.ascontiguousarray(alpha, dtype=np.float32)

    B = features.shape[0]
    assert B == B_CORE * N_CORES, B
    in_maps = [
        {"features": features[i * B_CORE:(i + 1) * B_CORE], "theta": theta,
         "alpha": alpha}
        for i in range(N_CORES)
    ]
    res = run_bass_kernel_spmd(nc, in_maps, core_ids=list(range(N_CORES)))
    LAST_RESULTS = res
    return np.concatenate([r["out"] for r in res.results], axis=0)


if __name__ == "__main__":
    feats = np.random.rand(1024, 12).astype(np.float32)
    th = (0.01 * np.random.randn(4, 12, 3)).astype(np.float32)
    al = np.zeros(4, np.float32)
    y = kernel(feats, th, al)
    print(y.shape, y.dtype, y[:2])


# revision 34
# speedup vs baseline: 1.1655x; 1.1655x over previous
"""Trainium2 Bass kernel for nn_EnsembleSharedVQC: 12-qubit, 4-layer VQC ensemble
(4 encoders, shared theta), batch 1024, <Z_q> readout, softmax(alpha) mixture.

Sharding: pure data parallelism, 8 cores x 128 samples; theta/alpha replicated.
Everything is SBUF-resident per core (state = 128x4096 re/im f32 planes);
HBM traffic is just the tiny inputs/outputs.

Pipeline (f32 numerics throughout):
  - T-major layout S[h, b*32+u], h = (q6, q0..q5) on partitions (q6 = MSB),
    u = (q7..q11) in the free dim.
  - Per layer l, on-device-built fused unitaries (identity evolved through the
    elementwise gate machinery, CNOTs folded):
      A_l = E_hi R_hi O_hi^{l-1} (128x128 complex, hi qubits),
      B_l = E_lo R_lo O_lo^{l-1} (32x32, replicated to blockdiag4).
  - Layer 0 folded into the encoding: the state is a product state H (x) L
    until the first crossing CNOT(6,7), so evolve the factors (H1 = A0 H,
    L1 = B0 L, L1x = X7 B0 L; all tiny matmuls) and materialize
    S[h,(b,u)] = H1[h,b] * Lsel(q6)[u,b] with a q6-conditional broadcast
    combine. Skips layer-0 MMs, transposes, and evacuations entirely.
  - Layers 1..3: hi-MM and lo-MM via Gauss 3-mult complex multiply
    (M1 = Ar S_re, M2 = Ai S_im, M3 = (Ar+Ai)(S_re+S_im); re = M1-M2,
    im = M3-M1-M2), sum-plane prep per chunk on gpsimd, PSUM->SBUF combine
    as 1 ACT copy + 3 DVE subs (one PSUM input per DVE op). The lo-MM
    applies blockdiag(B) on q6=0 columns and blockdiag(X7 B) on q6=1
    columns, consuming the crossing CNOT(6,7).
  - Transposes: 4 PE 128x128 transposes share one PSUM bank -> single
    [128,512] ACT evacuation copy.
  - Last layer: |psi|^2 computed in u-major (ACT squares + DVE add), only
    the single p plane is transposed back.
  - h_angle_rx encodes to a constant state -> its whole 4-layer evolution
    runs on dedicated [128,128] tiles, emitted first so its latency hides
    under encoder-1 PE work; its measurement is deferred to the tail.
  - Measurement: only the last layer's odd CNOTs pend -> factorized hi/lo
    XOR-parities; signed marginals via sliced reduces + subtract-folds,
    per-sample outputs via one PE transpose per sign pattern.
  - softmax(alpha) mixing on device; out = [128, 12] f32 per core.
"""

import numpy as np

import concourse.bass as bass
import concourse.bacc as bacc
import concourse.mybir as mybir
import concourse.tile as tile
from concourse.bass_utils import run_bass_kernel_spmd

AF = mybir.ActivationFunctionType
OP = mybir.AluOpType
F32 = mybir.dt.float32

N = 12
DIM = 1 << N
LAYERS = 4
B_CORE = 128
N_CORES = 8
ENCODERS = ["angle_rx", "angle_ry", "h_angle_rx", "h_angle_ry"]
INV_SQRT2 = float(1.0 / np.sqrt(2.0))
HALF_PI = float(np.pi / 2.0)

HI_ORDER = [6, 0, 1, 2, 3, 4, 5]
LO_ORDER = [7, 8, 9, 10, 11]
E_HI = [(0, 1), (2, 3), (4, 5)]
O_HI = [(1, 2), (3, 4), (5, 6)]
E_LO = [(8, 9), (10, 11)]
O_LO = [(7, 8), (9, 10)]


def _bitview(ap, nbits, fixed):
    """View a [P, 2**nbits] plane with some bit positions fixed."""
    names = [f"b{i}" for i in range(nbits)]
    pat = "p ({}) -> p {}".format(" ".join(names), " ".join(names))
    v = ap.rearrange(pat, **{n: 2 for n in names[:-1]})
    idx = [slice(None)] * (nbits + 1)
    for pos, val in fixed.items():
        idx[1 + pos] = slice(val, val + 1)
    return v[tuple(idx)]


def _small_gate(nc, cur, nxt, nbits, pos, rev, cw, npart):
    """Fused SU(2) gate on wire `pos` of an npart x 2**nbits state."""
    pr, pi = cur
    qr, qi = nxt
    cw = {k: v[:npart] for k, v in cw.items()}
    combos = [()]
    for _ in rev:
        combos = [c + (v,) for c in combos for v in (0, 1)]
    pr0 = _bitview(pr, nbits, {pos: 0})[:npart]
    pi0 = _bitview(pi, nbits, {pos: 0})[:npart]
    stt = nc.vector.scalar_tensor_tensor
    for out_pl, a, c1, c2, c3, c4 in (
        (qr, 0, cw['w'], cw['nx'], cw['y'], cw['nz']),
        (qi, 0, cw['x'], cw['w'], cw['z'], cw['y']),
        (qr, 1, cw['ny'], cw['nz'], cw['w'], cw['x']),
        (qi, 1, cw['z'], cw['ny'], cw['nx'], cw['w']),
    ):
        o_full = _bitview(out_pl, nbits, {pos: a})[:npart]
        nc.scalar.mul(o_full, pr0, c1)
        stt(o_full, pi0, c2, o_full, op0=OP.mult, op1=OP.add)
        for combo in combos:
            ofix = {pos: a}
            ifix = {pos: 1}
            for rp, v in zip(rev, combo):
                ofix[rp] = v
                ifix[rp] = 1 - v
            o_p = _bitview(out_pl, nbits, ofix)[:npart]
            pr1 = _bitview(pr, nbits, ifix)[:npart]
            pi1 = _bitview(pi, nbits, ifix)[:npart]
            stt(o_p, pr1, c3, o_p, op0=OP.mult, op1=OP.add)
            stt(o_p, pi1, c4, o_p, op0=OP.mult, op1=OP.add)


def _small_swap(nc, planes, nbits, c_pos, t_pos, tmp, npart):
    """Physical CNOT(c_pos -> t_pos) swap on an npart x 2**nbits state."""
    qdim = 1 << (nbits - 2)
    for pl in planes:
        v0 = _bitview(pl, nbits, {c_pos: 1, t_pos: 0})[:npart]
        v1 = _bitview(pl, nbits, {c_pos: 1, t_pos: 1})[:npart]
        t = tmp[:npart, 0:qdim]
        nc.vector.tensor_copy(t, v0)
        nc.scalar.copy(v0, v1)
        nc.vector.tensor_copy(v1, t)


def build_nc_stage3():
    nc = bacc.Bacc(None, target_bir_lowering=False, debug=False)

    features = nc.dram_tensor("features", [B_CORE, N], F32, kind="ExternalInput").ap()
    theta = nc.dram_tensor("theta", [LAYERS, N, 3], F32, kind="ExternalInput").ap()
    alpha = nc.dram_tensor("alpha", [4], F32, kind="ExternalInput").ap()
    out = nc.dram_tensor("out", [B_CORE, N], F32, kind="ExternalOutput").ap()

    P = B_CORE
    AX = mybir.AxisListType

    from concourse.masks import make_identity

    with tile.TileContext(nc) as tc:
        with (
            tc.tile_pool(name="state", bufs=1) as sp,
            tc.tile_pool(name="small", bufs=1) as small,
            tc.tile_pool(name="scratch", bufs=4) as scratch,
            tc.tile_pool(name="mm", bufs=6, space="PSUM") as mmpool,
            tc.tile_pool(name="tp", bufs=2, space="PSUM") as tppool,
            tc.tile_pool(name="dram", bufs=8, space="DRAM") as dpool,
        ):
            # ---------- input DMA ----------
            feat = small.tile([P, N], F32, tag="feat", name="feat")
            nc.sync.dma_start(out=feat, in_=features)
            th = small.tile([P, LAYERS, N, 3], F32, tag="th", name="th")
            th_b = bass.AP(tensor=theta.tensor, offset=0,
                           ap=[[0, P], [N * 3, LAYERS], [3, N], [1, 3]])
            nc.sync.dma_start(out=th, in_=th_b)
            alp = small.tile([P, 4], F32, tag="alp", name="alp")
            alp_b = bass.AP(tensor=alpha.tensor, offset=0, ap=[[0, P], [1, 4]])
            nc.sync.dma_start(out=alp, in_=alp_b)

            # ---------- trig ----------
            hpi = small.tile([P, 1], F32, tag="hpi", name="hpi")
            nc.vector.memset(hpi, HALF_PI)
            fh = small.tile([P, N], F32, tag="fh", name="fh")
            nc.scalar.activation(fh, feat, AF.Copy, scale=0.5)
            cf = small.tile([P, N], F32, tag="cf", name="cf")
            nc.scalar.activation(cf, fh, AF.Sin, bias=hpi)
            sf = small.tile([P, N], F32, tag="sf", name="sf")
            nc.scalar.activation(sf, fh, AF.Sin)
            nsf = small.tile([P, N], F32, tag="nsf", name="nsf")
            nc.vector.tensor_scalar_mul(nsf, sf, -1.0)
            hc = small.tile([P, N], F32, tag="hc", name="hc")
            nc.vector.tensor_sub(hc, cf, sf)
            nc.vector.tensor_scalar_mul(hc, hc, INV_SQRT2)
            hs = small.tile([P, N], F32, tag="hs", name="hs")
            nc.vector.tensor_add(hs, cf, sf)
            nc.vector.tensor_scalar_mul(hs, hs, INV_SQRT2)

            def flat(ap):
                return ap.rearrange("p a b c -> p (a b c)")

            thh = small.tile([P, LAYERS, N, 3], F32, tag="thh", name="thh")
            nc.scalar.activation(flat(thh), flat(th), AF.Copy, scale=0.5)
            ct = small.tile([P, LAYERS, N, 3], F32, tag="ct", name="ct")
            nc.scalar.activation(flat(ct), flat(thh), AF.Sin, bias=hpi)
            st = small.tile([P, LAYERS, N, 3], F32, tag="st", name="st")
            nc.scalar.activation(flat(st), flat(thh), AF.Sin)

            ca, cb, cg = (ct[:, :, :, i:i + 1] for i in range(3))
            sa, sb, sg = (st[:, :, :, i:i + 1] for i in range(3))

            def lq_tile(tag):
                return small.tile([P, LAYERS, N, 1], F32, tag=tag, name=tag)

            t1, t2, t3, t4 = (lq_tile(f"t{i}") for i in range(4))
            nc.vector.tensor_mul(t1, cg, cb)
            nc.vector.tensor_mul(t2, sg, sb)
            nc.vector.tensor_mul(t3, cg, sb)
            nc.vector.tensor_mul(t4, sg, cb)
            u1, u2 = lq_tile("u1"), lq_tile("u2")
            w_c, x_c, y_c, z_c = (lq_tile(t) for t in ("w", "x", "y", "z"))
            nx_c, ny_c, nz_c = (lq_tile(t) for t in ("nx", "ny", "nz"))
            nc.vector.tensor_mul(u1, t1, ca)
            nc.vector.tensor_mul(u2, t2, sa)
            nc.vector.tensor_add(w_c, u1, u2)
            nc.vector.tensor_mul(u1, t3, sa)
            nc.vector.tensor_mul(u2, t4, ca)
            nc.vector.tensor_sub(x_c, u1, u2)
            nc.vector.tensor_scalar_mul(nx_c, x_c, -1.0)
            nc.vector.tensor_mul(u1, t3, ca)
            nc.vector.tensor_mul(u2, t4, sa)
            nc.vector.tensor_add(ny_c, u1, u2)
            nc.vector.tensor_scalar_mul(y_c, ny_c, -1.0)
            nc.vector.tensor_mul(u1, t2, ca)
            nc.vector.tensor_mul(u2, t1, sa)
            nc.vector.tensor_sub(z_c, u1, u2)
            nc.vector.tensor_scalar_mul(nz_c, z_c, -1.0)

            def coefs(l, q):
                return {k: c[:, l:l + 1, q:q + 1, :] for k, c in
                        (('w', w_c), ('x', x_c), ('y', y_c), ('z', z_c),
                         ('nx', nx_c), ('ny', ny_c), ('nz', nz_c))}

            # ---------- softmax(alpha) ----------
            amax = small.tile([P, 1], F32, tag="amax", name="amax")
            nc.vector.reduce_max(amax, alp, axis=AX.X)
            esh = small.tile([P, 4], F32, tag="esh", name="esh")
            nc.vector.tensor_scalar(esh, alp, amax, None, op0=OP.subtract)
            nc.scalar.activation(esh, esh, AF.Exp)
            ssum = small.tile([P, 1], F32, tag="ssum", name="ssum")
            nc.vector.reduce_sum(ssum, esh, axis=AX.X)
            rsum = small.tile([P, 1], F32, tag="rsum", name="rsum")
            nc.vector.reciprocal(rsum, ssum)
            wts = small.tile([P, 4], F32, tag="wts", name="wts")
            nc.vector.tensor_scalar(wts, esh, rsum, None, op0=OP.mult)

            # ---------- identity for PE transposes ----------
            ident = small.tile([P, P], F32, tag="ident", name="ident")
            make_identity(nc, ident)

            # ---------- build A_l (hi) and B_l (lo) stationaries ----------
            bre_a = small.tile([P, P], F32, tag="bre_a", name="bre_a")
            bim_a = small.tile([P, P], F32, tag="bim_a", name="bim_a")
            bre_b = small.tile([P, P], F32, tag="bre_b", name="bre_b")
            bim_b = small.tile([P, P], F32, tag="bim_b", name="bim_b")
            btmp = small.tile([P, 64], F32, tag="btmp", name="btmp")

            A_t = []   # per layer: (Ar, Ai, Aq=Ar+Ai) [128,128] lhsT (= A^T)
            B_t = []   # per layer: (Br, Bi, Bq, BrX, BiX, BqX) blockdiag lhsT
            sB0 = {}   # layer-0 32x32 lo lhsT for the factorized layer 0

            def build_small(order, qubits, flips_pre, flips_post, l, npart):
                nbits = {7: 7, 5: 5}[len(order)]
                dim = 1 << nbits
                make_identity(nc, bre_a[:dim, :dim])
                nc.gpsimd.memset(bim_a[:, 0:dim], 0.0)
                cur = (bre_a[:, 0:dim], bim_a[:, 0:dim])
                nxt = (bre_b[:, 0:dim], bim_b[:, 0:dim])
                pend = [(order.index(c), order.index(t)) for c, t in flips_pre]
                for q in qubits:
                    pos = order.index(q)
                    rev = [t for c_, t in pend if c_ == pos]
                    pend = [(c_, t) for c_, t in pend if c_ != pos]
                    _small_gate(nc, cur, nxt, nbits, pos, rev, coefs(l, q), npart)
                    cur, nxt = nxt, cur
                assert not pend
                for c_, t in flips_post:
                    _small_swap(nc, cur, nbits, order.index(c_),
                                order.index(t), btmp, npart)
                return cur

            for l in range(LAYERS):
                ohi = O_HI if l > 0 else []
                olo = O_LO if l > 0 else []
                curA = build_small(HI_ORDER, [0, 1, 2, 3, 4, 5, 6], ohi, E_HI, l, P)
                Ar = small.tile([P, P], F32, tag=f"Ar{l}", name=f"Ar{l}")
                Ai = small.tile([P, P], F32, tag=f"Ai{l}", name=f"Ai{l}")
                Aq = small.tile([P, P], F32, tag=f"Aq{l}", name=f"Aq{l}")
                nc.vector.tensor_copy(Ar, curA[0])
                nc.vector.tensor_copy(Ai, curA[1])
                nc.vector.tensor_add(Aq, curA[0], curA[1])
                A_t.append((Ar, Ai, Aq))

                curB = build_small(LO_ORDER, [7, 8, 9, 10, 11], olo, E_LO, l, 32)
                sBr = small.tile([32, 32], F32, tag=f"sBr{l}", name=f"sBr{l}")
                sBi = small.tile([32, 32], F32, tag=f"sBi{l}", name=f"sBi{l}")
                sBq = small.tile([32, 32], F32, tag=f"sBq{l}", name=f"sBq{l}")
                nc.vector.tensor_copy(sBr, curB[0][:32, :32])
                nc.vector.tensor_copy(sBi, curB[1][:32, :32])
                nc.vector.tensor_add(sBq, curB[0][:32, :32], curB[1][:32, :32])
                if l == 0:
                    sBxr = small.tile([32, 32], F32, tag="sBxr", name="sBxr")
                    sBxi = small.tile([32, 32], F32, tag="sBxi", name="sBxi")
                    for dst, src in ((sBxr, sBr), (sBxi, sBi)):
                        nc.vector.tensor_copy(
                            dst.rearrange("p (t r) -> p t r", t=2),
                            src.rearrange("p (t r) -> p t r", t=2)[:, ::-1, :])
                    sB0 = {"r": sBr, "i": sBi, "xr": sBxr, "xi": sBxi}
                names = (f"Br{l}", f"Bi{l}", f"Bq{l}",
                         f"BrX{l}", f"BiX{l}", f"BqX{l}")
                tiles = []
                for nm in names:
                    tt = small.tile([P, P], F32, tag=nm, name=nm)
                    nc.gpsimd.memset(tt, 0.0)
                    tiles.append(tt)
                Br, Bi, Bq, BrX, BiX, BqX = tiles
                for i in range(4):
                    sl = slice(32 * i, 32 * i + 32)
                    for dst, src in ((Br, sBr), (Bi, sBi), (Bq, sBq)):
                        nc.sync.dma_start(out=dst[sl, sl], in_=src)
                    for dst, src in ((BrX, sBr), (BiX, sBi), (BqX, sBq)):
                        swp = dst[sl, sl].rearrange("p (t r) -> p t r", t=2)
                        nc.sync.dma_start(
                            out=swp, in_=src.rearrange(
                                "p (t r) -> p t r", t=2)[:, ::-1, :])
                B_t.append((Br, Bi, Bq, BrX, BiX, BqX))

            # ---------- state planes ----------
            S_re = sp.tile([P, DIM], F32, tag="S_re", name="S_re")
            S_im = sp.tile([P, DIM], F32, tag="S_im", name="S_im")
            T_re = sp.tile([P, DIM], F32, tag="T_re", name="T_re")
            T_im = sp.tile([P, DIM], F32, tag="T_im", name="T_im")
            U_re = sp.tile([P, DIM], F32, tag="U_re", name="U_re")
            U_im = sp.tile([P, DIM], F32, tag="U_im", name="U_im")
            V_re = sp.tile([P, DIM], F32, tag="V_re", name="V_re")
            V_im = sp.tile([P, DIM], F32, tag="V_im", name="V_im")
            Ssum = sp.tile([P, DIM], F32, tag="Ssum", name="Ssum")

            # encoding scratch
            Hb_re = small.tile([P, P], F32, tag="Hb_re", name="Hb_re")
            Hb_im = small.tile([P, P], F32, tag="Hb_im", name="Hb_im")
            H_re = small.tile([P, P], F32, tag="H_re", name="H_re")
            H_im = small.tile([P, P], F32, tag="H_im", name="H_im")
            H1_re = small.tile([P, P], F32, tag="H1_re", name="H1_re")
            H1_im = small.tile([P, P], F32, tag="H1_im", name="H1_im")
            Lb_re = small.tile([P, 32], F32, tag="Lb_re", name="Lb_re")
            Lb_im = small.tile([P, 32], F32, tag="Lb_im", name="Lb_im")
            Lu_re = small.tile([32, P], F32, tag="Lu_re", name="Lu_re")
            Lu_im = small.tile([32, P], F32, tag="Lu_im", name="Lu_im")
            L1 = {}
            for nm in ("re", "im", "xre", "xim"):
                L1[nm] = small.tile([P, 32], F32, tag=f"L1{nm}", name=f"L1{nm}")

            zacc = small.tile([P, N], F32, tag="zacc", name="zacc")
            nc.vector.memset(zacc, 0.0)

            def doubling(re_t, im_t, order, enc, cplx):
                nc.vector.memset(re_t[:, 0:1], 1.0)
                if cplx:
                    nc.gpsimd.memset(im_t, 0.0)
                size = 1
                for q in reversed(order):
                    lo = re_t[:, 0:size]
                    hi = re_t[:, size:2 * size]
                    if enc == "angle_rx":
                        loi = im_t[:, 0:size]
                        hii = im_t[:, size:2 * size]
                        v0 = cf[:, q:q + 1]
                        nc.scalar.mul(hi, loi, sf[:, q:q + 1])
                        nc.scalar.mul(hii, lo, nsf[:, q:q + 1])
                        nc.scalar.mul(loi, loi, v0)
                        nc.scalar.mul(lo, lo, v0)
                    else:
                        if enc == "angle_ry":
                            a_ap, b_ap = cf[:, q:q + 1], sf[:, q:q + 1]
                        else:
                            a_ap, b_ap = hc[:, q:q + 1], hs[:, q:q + 1]
                        nc.scalar.mul(hi, lo, b_ap)
                        nc.scalar.mul(lo, lo, a_ap)
                    size *= 2

            def u_major_view(plane, g0, ng):
                """[p, 2(q6), ng, 64] view of U-major cols, groups g0..g0+ng."""
                v = plane.rearrange("p (s g2 h) -> p s g2 h", s=2, g2=32)
                return v[:, :, g0:g0 + ng, :]

            def gauss_mm(dst_re, dst_im, Gr, Gi, Gq, src_re, src_im, src_q,
                         width, mview=None):
                """dst = G @ src (complex) via Gauss; dst views [128,width].
                mview reshapes the PSUM banks to match scattered dst views."""
                m1 = mmpool.tile([P, width], F32, tag="mm", name="m1")
                m2 = mmpool.tile([P, width], F32, tag="mm", name="m2")
                m3 = mmpool.tile([P, width], F32, tag="mm", name="m3")
                nc.tensor.matmul(m1, Gr, src_re, start=True, stop=True)
                nc.tensor.matmul(m2, Gi, src_im, start=True, stop=True)
                nc.tensor.matmul(m3, Gq, src_q, start=True, stop=True)
                v1, v2, v3 = ((mview(m1), mview(m2), mview(m3)) if mview
                              else (m1, m2, m3))
                # only one PSUM input per DVE op: stage M1 into dst_re first
                nc.scalar.copy(dst_re, v1)
                nc.vector.tensor_sub(dst_im, v3, dst_re)
                nc.vector.tensor_sub(dst_im, dst_im, v2)
                nc.vector.tensor_sub(dst_re, dst_re, v2)

            def measure(enc_i, p_t):
                """<Z_q> of the pending-flip-factorized p plane -> zacc."""
                LO_PATS = {0: (), 1: (0,), 2: (0, 1), 3: (2,), 4: (2, 3),
                           5: (4,)}
                r_pats = {}
                for pid, bits in LO_PATS.items():
                    eng = nc.vector
                    if not bits:
                        r = small.tile([P, P], F32, tag=f"rpat{pid}",
                                       name=f"rp{pid}")
                        eng.reduce_sum(
                            r, p_t.rearrange("p (b u) -> p b u", u=32),
                            axis=AX.X)
                        r_pats[pid] = r
                        continue
                    a0, b0 = bits[0], bits[-1]
                    runw = 1 << (b0 - a0 + 1)
                    o_sz = 1 << a0
                    i_sz = 32 // (o_sz * runw)
                    w4 = scratch.tile([P, P * runw], F32, tag="w4",
                                      name=f"w4_{pid}")
                    if o_sz == 1 and i_sz > 1:
                        vv = p_t.rearrange("p (b t i) -> p b t i", b=P, t=runw)
                        eng.reduce_sum(w4, vv, axis=AX.X)
                    elif i_sz == 1 and o_sz > 1:
                        vv = p_t.rearrange("p (b o t) -> p b t o", b=P, t=runw)
                        eng.reduce_sum(w4, vv, axis=AX.X)
                    else:
                        vv = p_t.rearrange("p (b o t i) -> p b t o i",
                                           b=P, o=o_sz, t=runw)
                        eng.reduce_sum(w4, vv, axis=AX.XY)
                    src, width = w4, runw
                    while width > 1:
                        width //= 2
                        dst = (scratch.tile([P, P * width], F32, tag="fold2",
                                            name="fold2")
                               if width > 1 else
                               small.tile([P, P], F32, tag=f"rpat{pid}",
                                          name=f"rpf{pid}"))
                        s2 = src.rearrange("p (b t) -> p b t", t=2 * width)
                        eng.tensor_sub(
                            dst.rearrange("p (b t) -> p b t", t=width),
                            s2[:, :, 0:width], s2[:, :, width:2 * width])
                        src = dst
                    r_pats[pid] = src

                rT = {}
                for pid, r in r_pats.items():
                    ptr = tppool.tile([P, 512], F32, tag="tp", name="ptr")
                    nc.tensor.transpose(ptr[:, 0:P], r, ident)
                    rt = small.tile([P, P], F32, tag=f"rT{pid}",
                                    name=f"rT{pid}")
                    nc.scalar.copy(rt, ptr[:, 0:P])
                    rT[pid] = rt

                z_e = small.tile([P, N], F32, tag=f"z_e{enc_i}",
                                 name=f"z_e{enc_i}")
                T_HI = {0: [0], 1: [1], 2: [1, 2], 3: [3], 4: [3, 4], 5: [5],
                        6: [5, 6]}
                for q in range(N):
                    if q <= 6:
                        src_m = rT[0]
                        bits = sorted(HI_ORDER.index(b2) for b2 in T_HI[q])
                        nb = 7
                    else:
                        pid = {7: 1, 8: 2, 9: 3, 10: 4, 11: 5}[q]
                        src_m = rT[pid]
                        bits = []
                        nb = 7
                    enq = nc.vector
                    if not bits:
                        enq.reduce_sum(z_e[:, q:q + 1], src_m, axis=AX.X)
                        continue
                    rest = [i for i in range(nb) if i not in bits]
                    runs = []
                    for i in rest:
                        if runs and runs[-1][-1] == i - 1:
                            runs[-1].append(i)
                        else:
                            runs.append([i])
                    assert len(runs) <= 2, (bits, runs)
                    names = [f"h{i}" for i in range(nb)]
                    pat = "p ({}) -> p {} {}".format(
                        " ".join(names),
                        " ".join(names[i] for i in bits),
                        " ".join("(" + " ".join(names[j] for j in run) + ")"
                                 for run in runs))
                    vv = src_m.rearrange(pat, **{n: 2 for n in names[:-1]})
                    kw = 1 << len(bits)
                    wq = scratch.tile([P, kw], F32, tag="wq", name="wq")
                    enq.reduce_sum(
                        wq, vv, axis=AX.X if len(runs) == 1 else AX.XY)
                    srcf, width = wq, kw
                    while width > 1:
                        width //= 2
                        dstf = (z_e[:, q:q + 1] if width == 1 else
                                scratch.tile([P, width], F32, tag="foldq",
                                             name="foldq"))
                        enq.tensor_sub(dstf, srcf[:, 0:width],
                                       srcf[:, width:2 * width])
                        srcf = dstf
                nc.vector.scalar_tensor_tensor(
                    zacc, z_e, wts[:, enc_i:enc_i + 1], zacc,
                    op0=OP.mult, op1=OP.add)

            # ---------- h_angle_rx branch on dedicated small tiles ----------
            # Sample-independent constant state: evolve one 128-col block.
            # Emitted first so its latency hides under encoder-1 PE work;
            # measurement is deferred to the tail.
            f3 = {nm: small.tile([P, 128], F32, tag=f"f3{nm}", name=f"f3{nm}")
                  for nm in ("Sre", "Sim", "Tre", "Tim", "Ure", "Uim",
                             "Vre", "Vim", "sum")}
            nc.vector.memset(f3["Sre"], float(2.0 ** -6))
            nc.gpsimd.memset(f3["Sim"], 0.0)
            for l in range(LAYERS):
                Ar, Ai, Aq = A_t[l]
                Br, Bi, Bq, BrX, BiX, BqX = B_t[l]
                nc.gpsimd.tensor_add(f3["sum"], f3["Sre"], f3["Sim"])
                gauss_mm(f3["Tre"], f3["Tim"], Ar, Ai, Aq,
                         f3["Sre"], f3["Sim"], f3["sum"], 128)
                for nin, nout in (("Tre", "Ure"), ("Tim", "Uim")):
                    ptf = tppool.tile([P, 512], F32, tag="tp", name="ptf3")
                    nc.tensor.transpose(ptf[:, 0:P], f3[nin], ident)
                    nc.scalar.copy(f3[nout], ptf[:, 0:P])
                nc.gpsimd.tensor_add(f3["sum"][:, 0:64], f3["Ure"][:, 0:64],
                                     f3["Uim"][:, 0:64])
                nc.gpsimd.tensor_add(f3["sum"][:, 64:128],
                                     f3["Ure"][:, 64:128],
                                     f3["Uim"][:, 64:128])
                for (ucs, br, bi, bq) in (
                    (slice(0, 64), Br, Bi, Bq),
                    (slice(64, 128), BrX, BiX, BqX),
                ):
                    gauss_mm(f3["Vre"][:, ucs], f3["Vim"][:, ucs], br, bi, bq,
                             f3["Ure"][:, ucs], f3["Uim"][:, ucs],
                             f3["sum"][:, ucs], 64)
                for nin, nout in (("Vre", "Sre"), ("Vim", "Sim")):
                    ptb = tppool.tile([P, 512], F32, tag="tp", name="ptb3")
                    nc.tensor.transpose(ptb[:, 0:P], f3[nin], ident)
                    nc.scalar.copy(f3[nout], ptb[:, 0:P])

            # ---------- the three data-dependent encoders ----------
            for enc, enc_i in (("angle_rx", 0), ("angle_ry", 1),
                               ("h_angle_ry", 3)):
                # ---------- factors + layer 0 ----------
                cplx = enc == "angle_rx"
                doubling(Hb_re, Hb_im, HI_ORDER, enc, cplx)
                doubling(Lb_re, Lb_im, LO_ORDER, enc, cplx)
                # Hb -> H (h-major)
                pt = tppool.tile([P, 512], F32, tag="tp", name="ptH")
                nc.tensor.transpose(pt[:, 0:P], Hb_re, ident)
                nc.scalar.copy(H_re, pt[:, 0:P])
                if cplx:
                    pt2 = tppool.tile([P, 512], F32, tag="tp", name="ptH2")
                    nc.tensor.transpose(pt2[:, 0:P], Hb_im, ident)
                    nc.scalar.copy(H_im, pt2[:, 0:P])
                # H1 = A0 @ H
                Ar0, Ai0, _Aq0 = A_t[0]
                mre = mmpool.tile([P, 512], F32, tag="mm", name="mre")
                mim = mmpool.tile([P, 512], F32, tag="mm", name="mim")
                if cplx:
                    m2b = mmpool.tile([P, 512], F32, tag="mm", name="m2b")
                    nc.tensor.matmul(mre[:, 0:P], Ar0, H_re,
                                     start=True, stop=True)
                    nc.tensor.matmul(m2b[:, 0:P], Ai0, H_im,
                                     start=True, stop=True)
                    nc.scalar.copy(H1_re, mre[:, 0:P])
                    nc.vector.tensor_sub(H1_re, H1_re, m2b[:, 0:P])
                    nc.tensor.matmul(mim[:, 0:P], Ar0, H_im,
                                     start=True, stop=False)
                    nc.tensor.matmul(mim[:, 0:P], Ai0, H_re,
                                     start=False, stop=True)
                    nc.vector.tensor_copy(H1_im, mim[:, 0:P])
                else:
                    nc.tensor.matmul(mre[:, 0:P], Ar0, H_re,
                                     start=True, stop=True)
                    nc.tensor.matmul(mim[:, 0:P], Ai0, H_re,
                                     start=True, stop=True)
                    nc.scalar.copy(H1_re, mre[:, 0:P])
                    nc.vector.tensor_copy(H1_im, mim[:, 0:P])
                # Lb -> Lu (u-major)
                ptl = tppool.tile([P, 512], F32, tag="tp", name="ptL")
                nc.tensor.transpose(ptl[:32, 0:P], Lb_re, ident)
                nc.scalar.copy(Lu_re, ptl[:32, 0:P])
                if cplx:
                    ptl2 = tppool.tile([P, 512], F32, tag="tp", name="ptL2")
                    nc.tensor.transpose(ptl2[:32, 0:P], Lb_im, ident)
                    nc.scalar.copy(Lu_im, ptl2[:32, 0:P])
                # L1b = (B0 L)^T, L1xb = (X7 B0 L)^T, directly b-major:
                # matmul(lhsT=Lu, rhs=sB0) = Lu.T @ B0^T = Lb @ B0^T
                for pre, keyr, keyi in (("", "r", "i"), ("x", "xr", "xi")):
                    br, bi = sB0[keyr], sB0[keyi]
                    ma = mmpool.tile([P, 512], F32, tag="mm", name="ma")
                    mb = mmpool.tile([P, 512], F32, tag="mm", name="mb")
                    if cplx:
                        mc = mmpool.tile([P, 512], F32, tag="mm", name="mc")
                        nc.tensor.matmul(ma[:, 0:32], Lu_re, br,
                                         start=True, stop=True)
                        nc.tensor.matmul(mc[:, 0:32], Lu_im, bi,
                                         start=True, stop=True)
                        nc.scalar.copy(L1[pre + "re"], ma[:, 0:32])
                        nc.vector.tensor_sub(L1[pre + "re"],
                                             L1[pre + "re"], mc[:, 0:32])
                        nc.tensor.matmul(mb[:, 0:32], Lu_im, br,
                                         start=True, stop=False)
                        nc.tensor.matmul(mb[:, 0:32], Lu_re, bi,
                                         start=False, stop=True)
                        nc.vector.tensor_copy(L1[pre + "im"], mb[:, 0:32])
                    else:
                        nc.tensor.matmul(ma[:, 0:32], Lu_re, br,
                                         start=True, stop=True)
                        nc.tensor.matmul(mb[:, 0:32], Lu_re, bi,
                                         start=True, stop=True)
                        nc.scalar.copy(L1[pre + "re"], ma[:, 0:32])
                        nc.vector.tensor_copy(L1[pre + "im"], mb[:, 0:32])
                # broadcast Lsel over partitions via DRAM roundtrip:
                # staging planes Ssum (re) / V_im (im) are free here;
                # dest[p=(q6,h6), (b,u)] = (L1 if q6=0 else L1x)[u, b]
                for nm, plane, half in (("re", Ssum, 0), ("im", V_im, 0),
                                        ("xre", Ssum, 1), ("xim", V_im, 1)):
                    dl = dpool.tile([P, 32], F32, tag=f"dl{nm}{half}",
                                    name=f"dl{nm}{half}")
                    nc.sync.dma_start(out=dl, in_=L1[nm])
                    rd = bass.AP(tensor=dl.tensor, offset=dl.offset,
                                 ap=[[0, 64], [1, DIM]])
                    nc.sync.dma_start(
                        out=plane[64 * half:64 * half + 64], in_=rd)

                # combine: S = hview(H1) * Lbc (complex); V_re as temp
                def hview(hp):
                    return hp.unsqueeze(2).broadcast_to((P, P, 32))
                sv_re = S_re.rearrange("p (b u) -> p b u", u=32)
                sv_im = S_im.rearrange("p (b u) -> p b u", u=32)
                lv_re = Ssum.rearrange("p (b u) -> p b u", u=32)
                lv_im = V_im.rearrange("p (b u) -> p b u", u=32)
                tv_re = V_re.rearrange("p (b u) -> p b u", u=32)
                nc.vector.tensor_tensor(sv_re, hview(H1_re), lv_re,
                                        op=OP.mult)
                nc.vector.tensor_tensor(tv_re, hview(H1_im), lv_im,
                                        op=OP.mult)
                nc.vector.tensor_sub(sv_re, sv_re, tv_re)
                nc.vector.tensor_tensor(sv_im, hview(H1_re), lv_im,
                                        op=OP.mult)
                nc.vector.tensor_tensor(tv_re, hview(H1_im), lv_re,
                                        op=OP.mult)
                nc.vector.tensor_add(sv_im, sv_im, tv_re)

                # ---------- layers 1..3 ----------
                for l in range(1, LAYERS):
                    lastp = l == LAYERS - 1
                    Ar, Ai, Aq = A_t[l]
                    Br, Bi, Bq, BrX, BiX, BqX = B_t[l]
                    # fused hi-MM+transpose: U_g = S_g^T A^T (Gauss),
                    # state chunks stationary, A^T tiles moving; output
                    # lands directly in U-major, no T stage at all.
                    for g0 in range(0, 32, 4):
                        m1 = mmpool.tile([P, 512], F32, tag="mm", name="m1")
                        m2 = mmpool.tile([P, 512], F32, tag="mm", name="m2")
                        m3 = mmpool.tile([P, 512], F32, tag="mm", name="m3")
                        for j in range(4):
                            gs = slice(128 * (g0 + j), 128 * (g0 + j) + 128)
                            js = slice(128 * j, 128 * j + 128)
                            nc.gpsimd.tensor_add(Ssum[:, gs], S_re[:, gs],
                                                 S_im[:, gs])
                            nc.tensor.matmul(m1[:, js], S_re[:, gs], Ar,
                                             start=True, stop=True)
                            nc.tensor.matmul(m2[:, js], S_im[:, gs], Ai,
                                             start=True, stop=True)
                            nc.tensor.matmul(m3[:, js], Ssum[:, gs], Aq,
                                             start=True, stop=True)

                        def uvw(m):
                            return m.rearrange("p (j s h) -> p s j h",
                                               j=4, s=2)
                        dst_re = u_major_view(U_re, g0, 4)
                        dst_im = u_major_view(U_im, g0, 4)
                        nc.scalar.copy(dst_re, uvw(m1))
                        nc.vector.tensor_sub(dst_im, uvw(m3), dst_re)
                        nc.vector.tensor_sub(dst_im, dst_im, uvw(m2))
                        nc.vector.tensor_sub(dst_re, dst_re, uvw(m2))
                    # lo-MM: V = blockdiag(B or X7 B) @ U (Gauss)
                    for c in range(8):
                        cs = slice(512 * c, 512 * c + 512)
                        nc.gpsimd.tensor_add(Ssum[:, cs], U_re[:, cs],
                                             U_im[:, cs])
                        br, bi, bq = (Br, Bi, Bq) if c < 4 else (BrX, BiX, BqX)
                        q6c = c // 4
                        gb = 8 * (c % 4)

                        def v_scat(plane):
                            v = plane.rearrange("p (g s h) -> p g s h",
                                                g=32, s=2)
                            return v[:, gb:gb + 8, q6c:q6c + 1, :]

                        def mvw(m):
                            return m.rearrange("p (g s h) -> p g s h",
                                               g=8, s=1)
                        gauss_mm(v_scat(V_re), v_scat(V_im), br, bi, bq,
                                 U_re[:, cs], U_im[:, cs], Ssum[:, cs], 512,
                                 mview=mvw)
                    if lastp:
                        # squares in u-major; transpose only the p plane
                        nc.scalar.activation(U_re, V_re, AF.Square)
                        nc.scalar.activation(U_im, V_im, AF.Square)
                        nc.vector.tensor_add(U_re, U_re, U_im)
                        for g0 in range(0, 32, 4):
                            pt4 = tppool.tile([P, 512], F32, tag="tp",
                                              name="pt4p")
                            for j in range(4):
                                gs = slice(128 * (g0 + j),
                                           128 * (g0 + j) + 128)
                                nc.tensor.transpose(
                                    pt4[:, 128 * j:128 * j + 128],
                                    U_re[:, gs], ident)
                            nc.scalar.copy(T_re[:, 512 * (g0 // 4):
                                                512 * (g0 // 4) + 512], pt4)
                        continue
                    # transpose back V -> S (batches of 4)
                    for g0 in range(0, 32, 4):
                        for pl_in, pl_out in ((V_re, S_re), (V_im, S_im)):
                            pt4 = tppool.tile([P, 512], F32, tag="tp",
                                              name="pt4b")
                            for j in range(4):
                                gs = slice(128 * (g0 + j), 128 * (g0 + j) + 128)
                                nc.tensor.transpose(
                                    pt4[:, 128 * j:128 * j + 128],
                                    pl_in[:, gs], ident)
                            nc.scalar.copy(
                                pl_out[:, 512 * (g0 // 4):
                                       512 * (g0 // 4) + 512], pt4)

                # ---------- measurement ----------
                measure(enc_i, T_re)  # p plane written by the lastp branch

            # ---------- deferred h_angle_rx measurement ----------
            f3p = small.tile([P, 32], F32, tag="f3p", name="f3p")
            f3q = small.tile([P, 32], F32, tag="f3q", name="f3q")
            nc.scalar.activation(f3p, f3["Sre"][:, 0:32], AF.Square)
            nc.scalar.activation(f3q, f3["Sim"][:, 0:32], AF.Square)
            nc.vector.tensor_add(f3p, f3p, f3q)
            nc.vector.tensor_copy(
                T_re.rearrange("p (b u) -> p b u", u=32),
                f3p.unsqueeze(1).broadcast_to((P, 128, 32)))
            measure(2, T_re)

            nc.sync.dma_start(out=out, in_=zacc)

    nc.finalize()
    return nc


_NC_CACHE = None
LAST_RESULTS = None  # BassKernelResults of the most recent run (for profiling)


def kernel(features: np.ndarray, theta: np.ndarray, alpha: np.ndarray) -> np.ndarray:
    global _NC_CACHE, LAST_RESULTS
    if _NC_CACHE is None:
        _NC_CACHE = build_nc_stage3()
    nc = _NC_CACHE

    features = np.ascontiguousarray(features, dtype=np.float32)
    theta = np.ascontiguousarray(theta, dtype=np.float32)
    alpha = np.ascontiguousarray(alpha, dtype=np.float32)

    B = features.shape[0]
    assert B == B_CORE * N_CORES, B
    in_maps = [
        {"features": features[i * B_CORE:(i + 1) * B_CORE], "theta": theta,
         "alpha": alpha}
        for i in range(N_CORES)
    ]
    res = run_bass_kernel_spmd(nc, in_maps, core_ids=list(range(N_CORES)))
    LAST_RESULTS = res
    return np.concatenate([r["out"] for r in res.results], axis=0)


if __name__ == "__main__":
    feats = np.random.rand(1024, 12).astype(np.float32)
    th = (0.01 * np.random.randn(4, 12, 3)).astype(np.float32)
    al = np.zeros(4, np.float32)
    y = kernel(feats, th, al)
    print(y.shape, y.dtype, y[:2])
